# revision 1
# baseline (speedup 1.0000x reference)
"""Trainium2 Bass kernel for nn_CausalSelfAttentionSynapse.

Math (per reference):
    qk = g @ W_lift.T                       # (B,T,2E)
    q,k heads of dim D=64; scores = q@k.T causal-masked
    lse[b,h,t] = logsumexp_{j<=t} scores[b,h,t,j]
    out[b,t]  = sum_h lse[b,h,t] * w[h],  w[h] = sum_g W_proj[g,h]

Sharding: 8 cores = 4 batches x 2 head-groups (8 heads each).
Per core (all on one NeuronCore):
  - PE-transpose g[b] and W_slice (head-group rows of W_lift) to get
    e-major layouts; lift matmuls (float32r, full PE rate) produce
    qT/kT (d on partitions) directly.
  - Per head pair, causal scores via 64x128 PE row tiling (2 heads
    run concurrently on the array), written to PSUM.
  - ScalarE exp (PSUM -> SBUF bf16), VectorE applies the triangular
    mask on the diagonal 128x128 block (multiplicative) and row-sums.
  - Ln + weighted head-sum on device; host adds the two head-group
    partials per batch.
"""

import numpy as np
import ml_dtypes

B, T, E, H = 4, 2048, 1024, 16
D = 64
NCORES = 8
NT = T // 128  # 16 g row tiles
NE = E // 128  # 8 e chunks
PAIRS = 4      # head pairs per core

_CACHE = {}


def _build():
    import concourse.bass as bass  # noqa: F401
    import concourse.tile as tile
    from concourse import bacc, mybir

    f32 = mybir.dt.float32
    f32r = mybir.dt.float32r
    bf16 = mybir.dt.bfloat16
    i32 = mybir.dt.int32
    EXP = mybir.ActivationFunctionType.Exp
    LN = mybir.ActivationFunctionType.Ln
    AX = mybir.AxisListType.X
    MUL = mybir.AluOpType.mult
    ADD = mybir.AluOpType.add
    SHR = mybir.AluOpType.logical_shift_right
    SUB = mybir.AluOpType.subtract
    AND = mybir.AluOpType.bitwise_and
    OR = mybir.AluOpType.bitwise_or
    LN2 = float(np.log(2.0))

    nc = bacc.Bacc("TRN2", target_bir_lowering=False, debug=False,
                   num_devices=NCORES)

    g_d = nc.dram_tensor("g_b", [T, E], f32, kind="ExternalInput").ap()
    w_d = nc.dram_tensor("w_slice", [E, E], f32, kind="ExternalInput").ap()
    wp_d = nc.dram_tensor("wp_cols", [16, 8], f32, kind="ExternalInput").ap()
    id_d = nc.dram_tensor("ident", [128, 128], f32, kind="ExternalInput").ap()
    tri_d = nc.dram_tensor("tri", [128, 128], bf16,
                           kind="ExternalInput").ap()
    out_d = nc.dram_tensor("out_part", [128, 16], f32,
                           kind="ExternalOutput").ap()

    with tile.TileContext(nc) as tc:
        with (
            tc.tile_pool(name="consts", bufs=1) as consts,
            tc.tile_pool(name="big", bufs=1) as big,
            tc.tile_pool(name="stage", bufs=3) as stage,
            tc.tile_pool(name="qkp", bufs=2) as qkp,
            tc.tile_pool(name="exps", bufs=3) as exps,
            tc.tile_pool(name="misc", bufs=1) as misc,
            tc.tile_pool(name="ps", bufs=4, space="PSUM") as ps,
        ):
            # ---- constants -------------------------------------------------
            ident = consts.tile([128, 128], f32, name="ident", tag="ident")
            nc.sync.dma_start(out=ident[:], in_=id_d[:])
            tri = consts.tile([128, 128], bf16, name="tri", tag="tri")
            nc.sync.dma_start(out=tri[:], in_=tri_d[:])
            wp = consts.tile([16, 8], f32, name="wp", tag="wp")
            nc.sync.dma_start(out=wp[:], in_=wp_d[:])
            ones16 = consts.tile([16, 128], f32, name="ones16", tag="ones16")
            nc.vector.memset(ones16[:], 1.0)

            # w[h] broadcast to all partitions: (128, 8)
            pw = ps.tile([128, 1024], f32, name="pw", tag="ps")
            nc.tensor.matmul(pw[:, 0:8], lhsT=ones16[:], rhs=wp[:],
                             start=True, stop=True)
            wb = consts.tile([128, 8], f32, name="wb", tag="wb")
            nc.vector.tensor_copy(wb[:], pw[:, 0:8])

            # ---- big SBUF layouts (float32r: rounded for PE full-rate) ----
            # gT[:, e*T + t]  : g transposed, e-chunk major
            gT = big.tile([128, NE * T], f32r, name="gT", tag="gT")
            # wT[:, e*E + f]  : W_slice transposed
            wT = big.tile([128, NE * E], f32r, name="wT", tag="wT")

            gT3 = gT.rearrange("p (e t) -> p e t", e=NE)
            wT3 = wT.rearrange("p (e f) -> p e f", e=NE)

            def g_transpose(ti):
                gst = stage.tile([128, E], f32, name=f"gst{ti}", tag="gst")
                nc.sync.dma_start(out=gst[:],
                                  in_=g_d[ti * 128:(ti + 1) * 128, :])
                pt = ps.tile([128, 1024], f32, name=f"ptg{ti}", tag="ps")
                for e in range(NE):
                    nc.tensor.transpose(pt[:, e * 128:(e + 1) * 128],
                                        gst[:, e * 128:(e + 1) * 128],
                                        ident[:])
                src = pt.rearrange("p (e t) -> p e t", e=NE)
                nc.vector.tensor_copy(
                    gT3[:, :, ti * 128:(ti + 1) * 128], src)

            def w_transpose(fi):
                wst = stage.tile([128, E], f32, name=f"wst{fi}", tag="wst")
                nc.sync.dma_start(out=wst[:],
                                  in_=w_d[fi * 128:(fi + 1) * 128, :])
                pt = ps.tile([128, 1024], f32, name=f"ptw{fi}", tag="ps")
                for e in range(NE):
                    nc.tensor.transpose(pt[:, e * 128:(e + 1) * 128],
                                        wst[:, e * 128:(e + 1) * 128],
                                        ident[:])
                src = pt.rearrange("p (e f) -> p e f", e=NE)
                nc.vector.tensor_copy(
                    wT3[:, :, fi * 128:(fi + 1) * 128], src)

            def lift_chunk(p, f, tcn, qkt):
                """Compute qkT for f-tile (2p+f), t-cols [tcn*1024, +1024)."""
                ft = 2 * p + f
                pt = ps.tile([128, 1024], f32, name=f"ptl{p}{f}{tcn}",
                             tag="ps")
                for e in range(NE):
                    lhsT = wT[:, e * E + ft * 128: e * E + ft * 128 + 128]
                    for half in range(2):
                        t0 = tcn * 1024 + half * 512
                        rhs = gT[:, e * T + t0: e * T + t0 + 512]
                        nc.tensor.matmul(
                            pt[:, half * 512:(half + 1) * 512],
                            lhsT=lhsT, rhs=rhs,
                            start=(e == 0), stop=(e == NE - 1))
                nc.vector.tensor_copy(
                    qkt[:, f * T + tcn * 1024: f * T + tcn * 1024 + 1024],
                    pt[:])

            sums_t = []
            for hh in range(8):
                st = misc.tile([128, 16], f32, name=f"sums{hh}", tag="sums",
                               bufs=8)
                sums_t.append(st)

            last_exp = [None]

            def scores_qtile(p, qi, qkt):
                """Causal scores + exp + fused mask/rowsum for both heads
                of pair p, query tile qi (rows qi*128..+128)."""
                kneed = 128 * (qi + 1)
                q0 = 128 * qi
                ntp = 1 if kneed <= 1024 else 2
                pst = [[ps.tile([128, 1024], f32,
                                name=f"pss{p}{qi}{h}{j}", tag="ps")
                        for j in range(ntp)] for h in range(2)]
                ko = 0
                while ko < kneed:
                    sz = min(512, kneed - ko)
                    j, off = ko // 1024, ko % 1024
                    for h in range(2):
                        lhsT = qkt[64 * h:64 * h + 64,
                                   qi * 128: qi * 128 + 128]
                        rhs = qkt[64 * h:64 * h + 64, T + ko: T + ko + sz]
                        nc.tensor.matmul(
                            pst[h][j][:, off:off + sz],
                            lhsT=lhsT, rhs=rhs,
                            start=True, stop=True)
                    ko += sz
                for h in range(2):
                    hh = 2 * p + h
                    eb = exps.tile([128, 2048], bf16,
                                   name=f"ebh{p}{qi}{h}", tag="eb")
                    for j in range(ntp):
                        cols = min(1024, kneed - 1024 * j)
                        nc.scalar.activation(
                            eb[:, 1024 * j:1024 * j + cols],
                            pst[h][j][:, 0:cols], EXP)
                    nc.vector.tensor_mul(
                        eb[:, kneed - 128:kneed],
                        eb[:, kneed - 128:kneed], tri[:])
                    nc.vector.reduce_sum(
                        out=sums_t[hh][:, qi:qi + 1],
                        in_=eb[:, 0:kneed], axis=AX)

            for fi in range(2):
                w_transpose(fi)
            for ti in range(8):
                g_transpose(ti)

            qkts = {}
            qkts[0] = qkp.tile([128, 2 * T], f32r, name="qkt0", tag="qkt")
            for f in range(2):
                lift_chunk(0, f, 0, qkts[0])
            for ti in range(8, 16):
                g_transpose(ti)
            # scores qi<8 only need keys 0:1024 (lift tc0) — start the
            # exp pipeline while the rest of the lift streams in
            for qi in range(4):
                scores_qtile(0, qi, qkts[0])
            for f in range(2):
                lift_chunk(0, f, 1, qkts[0])
            for qi in range(4, 8):
                scores_qtile(0, qi, qkts[0])
            for fi in range(2, 8):
                w_transpose(fi)
            for qi in range(8, 16):
                scores_qtile(0, qi, qkts[0])

            # pairs 1-3: interleave the NEXT pair's lift chunks between
            # qtiles so ACT/DVE never drain during lift phases
            qkts[1] = qkp.tile([128, 2 * T], f32r, name="qkt1", tag="qkt")
            for tcn in range(2):
                for f in range(2):
                    lift_chunk(1, f, tcn, qkts[1])
            for p in range(1, PAIRS):
                nxt = p + 1
                if nxt < PAIRS:
                    qkts[nxt] = qkp.tile([128, 2 * T], f32r,
                                         name=f"qkt{nxt}", tag="qkt")
                chunks = [(f, tcn) for tcn in range(2) for f in range(2)]
                ci = 0
                for qi in range(16):
                    scores_qtile(p, qi, qkts[p])
                    if nxt < PAIRS and qi % 4 == 3:
                        f, tcn = chunks[ci]
                        ci += 1
                        lift_chunk(nxt, f, tcn, qkts[nxt])

            # gate: forces the Ln chain to schedule after all exps are
            # done (keeps the ACT exp/ln table sets from thrashing)
            gate = misc.tile([128, 1], f32, name="gate", tag="gate")
            nc.vector.tensor_scalar(out=gate[:], in0=sums_t[7][:, 15:16],
                                    scalar1=0.0, scalar2=None, op0=MUL)

            # ---- finale: lse = ln(sums); out = sum_h w[h]*lse_h -----------
            # lse = ln(s) via exponent/mantissa split — ACT Ln is only
            # valid on ~[2^-66, 2^64] and sums span e^-49..e^56.
            # s = m * 2^(e-127), m in [1,2):
            #   lse = (e - 127)*ln2 + Ln(m)
            acc = [misc.tile([128, 16], f32, name=f"acc{i}", tag="acc",
                             bufs=2) for i in range(2)]
            nc.vector.memset(acc[0][:], 0.0)
            cur = 0
            for hh in range(8):
                u = sums_t[hh][:].bitcast(i32)
                ei = stage.tile([128, 16], i32, name=f"ei{hh}", tag="ei")
                nc.vector.tensor_scalar(out=ei[:], in0=u, scalar1=23,
                                        scalar2=None, op0=SHR)
                ef = stage.tile([128, 16], f32, name=f"ef{hh}", tag="ef")
                nc.vector.tensor_copy(ef[:], ei[:])
                nc.vector.tensor_scalar(out=ef[:], in0=ef[:], scalar1=127.0,
                                        scalar2=None, op0=SUB)
                mb = stage.tile([128, 16], i32, name=f"mb{hh}", tag="mb")
                nc.vector.tensor_scalar(out=mb[:], in0=u,
                                        scalar1=0x007FFFFF,
                                        scalar2=0x3F800000,
                                        op0=AND, op1=OR)
                lnm = stage.tile([128, 16], f32, name=f"lnm{hh}", tag="lnm")
                nc.scalar.activation(lnm[:], mb[:].bitcast(f32), LN,
                                     bias=gate[:, 0:1])
                lse = stage.tile([128, 16], f32, name=f"lse{hh}", tag="lse")
                nc.vector.scalar_tensor_tensor(
                    out=lse[:], in0=ef[:], scalar=LN2, in1=lnm[:],
                    op0=MUL, op1=ADD)
                nxt = 1 - cur
                nc.vector.scalar_tensor_tensor(
                    out=acc[nxt][:], in0=lse[:], scalar=wb[:, hh:hh + 1],
                    in1=acc[cur][:], op0=MUL, op1=ADD)
                cur = nxt
            nc.sync.dma_start(out=out_d[:], in_=acc[cur][:])

    nc.compile()
    return nc


def _get_nc():
    if "nc" not in _CACHE:
        _CACHE["nc"] = _build()
    return _CACHE["nc"]


def kernel(g, W_lift, W_proj):
    from concourse.bass_utils import run_bass_kernel_spmd

    g = np.asarray(g, dtype=np.float32)
    W_lift = np.asarray(W_lift, dtype=np.float32)
    W_proj = np.asarray(W_proj, dtype=np.float32)

    nc = _get_nc()
    ident = np.eye(128, dtype=np.float32)
    tri = np.tril(np.ones((128, 128), dtype=np.float32)).astype(
        ml_dtypes.bfloat16)

    in_maps = []
    for core in range(NCORES):
        b, hg = core // 2, core % 2
        rows = []
        for p in range(PAIRS):
            h0 = hg * 8 + 2 * p
            h1 = h0 + 1
            rows += list(range(h0 * D, h0 * D + D))
            rows += list(range(h1 * D, h1 * D + D))
            rows += list(range(E + h0 * D, E + h0 * D + D))
            rows += list(range(E + h1 * D, E + h1 * D + D))
        w_slice = np.ascontiguousarray(W_lift[rows, :])
        in_maps.append({
            "g_b": np.ascontiguousarray(g[b]),
            "w_slice": w_slice,
            "wp_cols": np.ascontiguousarray(W_proj[:, hg * 8:hg * 8 + 8]),
            "ident": ident,
            "tri": tri,
        })

    res = run_bass_kernel_spmd(nc, in_maps, core_ids=list(range(NCORES)))
    _CACHE["last_results"] = res
    _CACHE["last_in_maps"] = in_maps

    out = np.zeros((B, T), dtype=np.float32)
    for core in range(NCORES):
        b = core // 2
        part = res.results[core]["out_part"]  # (128, 16)
        out[b] += part.T.reshape(-1)
    return out



# revision 2
# speedup vs baseline: 1.1549x; 1.1549x over previous
"""Trainium2 Bass kernel for nn_CausalSelfAttentionSynapse (v2).

Math (per reference):
    qk = g @ W_lift.T; q,k heads of dim D=64
    lse[b,h,t] = logsumexp_{j<=t} (q_t . k_j)
    out[b,t]  = sum_h lse[b,h,t] * w[h],  w[h] = sum_g W_proj[g,h]

Sharding: 8 cores = 4 batches x 2 head-groups (8 heads each).

Per-core design (ScalarE-bound, everything else hidden under it):
  - Host pre-transposes g[b] and the head-group's W rows into e-major
    bf16 layouts (gT, wT) -> no on-device transposes at all.
  - Lift: q/k for each head pair via PE matmuls (bf16 in, fp32 PSUM),
    DVE-copied to SBUF bf16 (qkt tiles, d-major for the score matmuls).
  - Scores: per (head, q-tile of 128 rows) the causal row block
    [0, 128*(qi+1)) is matmul'd into a bank-aligned rotating PSUM ring;
    the causal mask of the diagonal 128x128 block is ADDED by one extra
    matmul (lhsT=I, rhs=strictly-upper -30000) accumulating into PSUM.
  - ONE ScalarE EXP per (head, q-tile) with accum_out producing that
    row-block's sum directly -- no DVE reduce anywhere.
  - Finale: lse = ln(sums) for all 128 (head, q-tile) columns at once
    via exponent/mantissa split (ACT Ln valid only on ~[2^-66, 2^64]),
    then weighted head-sum via a host-built replicated weight tile.
"""

import numpy as np
import ml_dtypes

B, T, E, H = 4, 2048, 1024, 16
D = 64
NCORES = 8
NE = 8         # 128-row chunks of e
PAIRS = 4      # head pairs per core
NQT = 16       # q tiles of 128 rows
MASKV = -30000.0

_CACHE = {}


def _build():
    import concourse.bass as bass  # noqa: F401
    import concourse.tile as tile
    from concourse import bacc, mybir

    f32 = mybir.dt.float32
    bf16 = mybir.dt.bfloat16
    i32 = mybir.dt.int32
    EXP = mybir.ActivationFunctionType.Exp
    LN = mybir.ActivationFunctionType.Ln
    MUL = mybir.AluOpType.mult
    ADD = mybir.AluOpType.add
    SHR = mybir.AluOpType.logical_shift_right
    SUB = mybir.AluOpType.subtract
    AND = mybir.AluOpType.bitwise_and
    OR = mybir.AluOpType.bitwise_or
    LN2 = float(np.log(2.0))

    nc = bacc.Bacc("TRN2", target_bir_lowering=False, debug=False,
                   num_devices=NCORES)

    # gt rows: half*1024 + e*128 + p ; cols: t-within-half
    g_d = nc.dram_tensor("gt", [T, E], bf16, kind="ExternalInput").ap()
    # wt rows: pair*1024 + e*128 + p ; cols: f-within-pair (q0 q1 k0 k1)
    w_d = nc.dram_tensor("wt", [4 * E, 256], bf16, kind="ExternalInput").ap()
    tri_d = nc.dram_tensor("tri", [128, 128], bf16, kind="ExternalInput").ap()
    id_d = nc.dram_tensor("identb", [128, 128], bf16,
                          kind="ExternalInput").ap()
    wr_d = nc.dram_tensor("wrep", [128, 128], f32, kind="ExternalInput").ap()
    out_d = nc.dram_tensor("out_part", [128, 16], f32,
                           kind="ExternalOutput").ap()

    with tile.TileContext(nc) as tc:
        with (
            tc.tile_pool(name="consts", bufs=1) as consts,
            tc.tile_pool(name="big", bufs=1) as big,
            tc.tile_pool(name="qkp", bufs=2) as qkp,
            tc.tile_pool(name="escp", bufs=2) as escp,
            tc.tile_pool(name="sums", bufs=1) as sums,
            tc.tile_pool(name="fin", bufs=1) as fin,
            tc.tile_pool(name="ps", bufs=1, space="PSUM") as ps,
        ):
            # ---- constants -------------------------------------------------
            tri = consts.tile([128, 128], bf16, name="tri", tag="tri")
            nc.sync.dma_start(out=tri[:], in_=tri_d[:])
            identb = consts.tile([128, 128], bf16, name="identb", tag="id")
            nc.sync.dma_start(out=identb[:], in_=id_d[:])
            wrep = consts.tile([128, 128], f32, name="wrep", tag="wrep")
            nc.sync.dma_start(out=wrep[:], in_=wr_d[:])

            # ---- big SBUF layouts (host-pretransposed, bf16) --------------
            # gT col = e*2048 + t ; wT col = e*1024 + pair*256 + fw
            gT = big.tile([128, NE * T], bf16, name="gT", tag="gT")
            wT = big.tile([128, NE * E], bf16, name="wT", tag="wT")

            # DMA order: pair-0 weights, g half0, g half1, pair-1..3 weights
            def dma_wt(pr):
                for e in range(NE):
                    nc.sync.dma_start(
                        out=wT[:, e * 1024 + pr * 256: e * 1024 + pr * 256 + 256],
                        in_=w_d[pr * 1024 + e * 128: pr * 1024 + e * 128 + 128, :])

            def dma_gt(half):
                for e in range(NE):
                    nc.sync.dma_start(
                        out=gT[:, e * 2048 + half * 1024:
                               e * 2048 + half * 1024 + 1024],
                        in_=g_d[half * 1024 + e * 128:
                                half * 1024 + e * 128 + 128, :])

            dma_wt(0)
            dma_gt(0)
            dma_gt(1)
            for pr in range(1, PAIRS):
                dma_wt(pr)

            # ---- PSUM ring (8 banks of 512 fp32, bank-aligned regions) ----
            ring = ps.tile([128, 4096], f32, name="ring", tag="ring")
            ring_pos = [0]

            def ring_alloc(nbanks):
                if ring_pos[0] + nbanks > 8:
                    ring_pos[0] = 0
                off = ring_pos[0] * 512
                ring_pos[0] += nbanks
                return off

            # sums[:, (2*pr+h)*16 + qi] = sum_j exp(scores) for that row tile
            sums1 = sums.tile([128, 128], f32, name="sums1", tag="sums")

            qkts = {}

            def lift_chunk(pr, ft, tcn):
                """qkt[pr] cols [ft*2048 + tcn*512, +512) from W f-tile."""
                off = ring_alloc(1)
                pt = ring[:, off:off + 512]
                w0 = pr * 256 + ft * 128
                for e in range(NE):
                    nc.tensor.matmul(
                        pt,
                        lhsT=wT[:, e * 1024 + w0: e * 1024 + w0 + 128],
                        rhs=gT[:, e * 2048 + tcn * 512:
                               e * 2048 + tcn * 512 + 512],
                        start=(e == 0), stop=(e == NE - 1))
                nc.vector.tensor_copy(
                    qkts[pr][:, ft * 2048 + tcn * 512:
                             ft * 2048 + tcn * 512 + 512], pt)

            def score_qtile(pr, qi):
                """Causal scores + fused exp/rowsum for both heads of pair
                pr, query rows [qi*128, +128)."""
                kneed = 128 * (qi + 1)
                nb = (kneed + 511) // 512
                qkt = qkts[pr]
                for h in range(2):
                    off = ring_alloc(nb)
                    lhsT = qkt[64 * h:64 * h + 64, qi * 128: qi * 128 + 128]
                    a = 0
                    while a < kneed:
                        sz = min(512, kneed - a)
                        last = (a + sz == kneed)
                        nc.tensor.matmul(
                            ring[:, off + a: off + a + sz],
                            lhsT=lhsT,
                            rhs=qkt[64 * h:64 * h + 64,
                                    2048 + a: 2048 + a + sz],
                            start=True, stop=not last)
                        a += sz
                    # additive causal mask on the diagonal block
                    nc.tensor.matmul(
                        ring[:, off + kneed - 128: off + kneed],
                        lhsT=identb[:], rhs=tri[:], start=False, stop=True)
                    esc = escp.tile([128, 2048], bf16,
                                    name=f"esc{pr}_{qi}_{h}", tag="esc")
                    col = (2 * pr + h) * 16 + qi
                    nc.scalar.activation(
                        esc[:, 0:kneed], ring[:, off: off + kneed], EXP,
                        accum_out=sums1[:, col:col + 1])

            # ---- schedule -------------------------------------------------
            qkts[0] = qkp.tile([128, 2 * T], bf16, name="qkt0", tag="qkt")
            lift_chunk(0, 0, 0)
            lift_chunk(0, 1, 0)
            p0_extra = [(0, 0, 1), (0, 1, 1), (0, 0, 2), (0, 1, 2),
                        (0, 0, 3), (0, 1, 3), None, None]
            for qi in range(8):
                score_qtile(0, qi)
                if p0_extra[qi] is not None:
                    lift_chunk(*p0_extra[qi])

            order = [(0, 0), (1, 0), (0, 1), (1, 1),
                     (0, 2), (1, 2), (0, 3), (1, 3)]
            for pr in range(PAIRS):
                if pr > 0:
                    for qi in range(8):
                        score_qtile(pr, qi)
                nxt = pr + 1
                if nxt < PAIRS:
                    qkts[nxt] = qkp.tile([128, 2 * T], bf16,
                                         name=f"qkt{nxt}", tag="qkt")
                for qi in range(8, 16):
                    score_qtile(pr, qi)
                    if nxt < PAIRS:
                        ft, tcn = order[qi - 8]
                        lift_chunk(nxt, ft, tcn)

            # ---- finale: lse = ln(sums); out = sum_h w[h]*lse_h -----------
            # s = m * 2^(e-127), m in [1,2):  lse = (e-127)*ln2 + Ln(m)
            u = sums1[:].bitcast(i32)
            ei = fin.tile([128, 128], i32, name="ei", tag="ei")
            nc.vector.tensor_scalar(out=ei[:], in0=u, scalar1=23,
                                    scalar2=None, op0=SHR)
            ef = fin.tile([128, 128], f32, name="ef", tag="ef")
            nc.vector.tensor_copy(ef[:], ei[:])
            nc.vector.tensor_scalar(out=ef[:], in0=ef[:], scalar1=127.0,
                                    scalar2=None, op0=SUB)
            mb = fin.tile([128, 128], i32, name="mb", tag="mb")
            nc.vector.tensor_scalar(out=mb[:], in0=u,
                                    scalar1=0x007FFFFF,
                                    scalar2=0x3F800000,
                                    op0=AND, op1=OR)
            lnm = fin.tile([128, 128], f32, name="lnm", tag="lnm")
            nc.scalar.activation(lnm[:], mb[:].bitcast(f32), LN)
            lse = fin.tile([128, 128], f32, name="lse", tag="lse")
            nc.vector.scalar_tensor_tensor(
                out=lse[:], in0=ef[:], scalar=LN2, in1=lnm[:],
                op0=MUL, op1=ADD)
            wl = fin.tile([128, 128], f32, name="wl", tag="wl")
            nc.vector.tensor_mul(wl[:], lse[:], wrep[:])
            h64 = fin.tile([128, 64], f32, name="h64", tag="h64")
            nc.vector.tensor_add(h64[:], wl[:, 0:64], wl[:, 64:128])
            h32 = fin.tile([128, 32], f32, name="h32", tag="h32")
            nc.vector.tensor_add(h32[:], h64[:, 0:32], h64[:, 32:64])
            facc = fin.tile([128, 16], f32, name="facc", tag="facc")
            nc.vector.tensor_add(facc[:], h32[:, 0:16], h32[:, 16:32])
            nc.sync.dma_start(out=out_d[:], in_=facc[:])

    nc.compile()
    return nc


def _get_nc():
    if "nc" not in _CACHE:
        _CACHE["nc"] = _build()
    return _CACHE["nc"]


def kernel(g, W_lift, W_proj):
    from concourse.bass_utils import run_bass_kernel_spmd

    bf16 = ml_dtypes.bfloat16
    g = np.asarray(g, dtype=np.float32)
    W_lift = np.asarray(W_lift, dtype=np.float32)
    W_proj = np.asarray(W_proj, dtype=np.float32)

    nc = _get_nc()
    w = W_proj.sum(axis=0).astype(np.float32)          # w[h] = sum_g W_proj[g,h]
    tri = np.triu(np.full((128, 128), MASKV, np.float32), k=1).astype(bf16)
    identb = np.eye(128, dtype=np.float32).astype(bf16)

    in_maps = []
    for core in range(NCORES):
        b, hg = core // 2, core % 2
        rows = []
        for p in range(PAIRS):
            h0 = hg * 8 + 2 * p
            h1 = h0 + 1
            rows += list(range(h0 * D, h0 * D + D))
            rows += list(range(h1 * D, h1 * D + D))
            rows += list(range(E + h0 * D, E + h0 * D + D))
            rows += list(range(E + h1 * D, E + h1 * D + D))
        W_slice = W_lift[rows, :]                      # [1024 f, 1024 e]
        wt_host = np.ascontiguousarray(
            W_slice.T.reshape(NE, 128, PAIRS, 256)
            .transpose(2, 0, 1, 3).reshape(4 * E, 256)).astype(bf16)
        gt_host = np.ascontiguousarray(
            g[b].reshape(2, 1024, NE, 128)
            .transpose(0, 2, 3, 1).reshape(T, E)).astype(bf16)
        wrep = np.ascontiguousarray(
            np.broadcast_to(np.repeat(w[hg * 8: hg * 8 + 8], 16)[None, :],
                            (128, 128))).astype(np.float32)
        in_maps.append({
            "gt": gt_host,
            "wt": wt_host,
            "tri": tri,
            "identb": identb,
            "wrep": wrep,
        })

    res = run_bass_kernel_spmd(nc, in_maps, core_ids=list(range(NCORES)))
    _CACHE["last_results"] = res
    _CACHE["last_in_maps"] = in_maps

    out = np.zeros((B, T), dtype=np.float32)
    for core in range(NCORES):
        b = core // 2
        part = res.results[core]["out_part"]           # (128, 16)
        out[b] += part.T.reshape(-1)
    return out


# revision 3
# speedup vs baseline: 1.2842x; 1.1120x over previous
"""Trainium2 Bass kernel for nn_CausalSelfAttentionSynapse (v3).

Math (per reference):
    qk = g @ W_lift.T; q,k heads of dim D=64
    lse[b,h,t] = logsumexp_{j<=t} (q_t . k_j)
    out[b,t]  = sum_h lse[b,h,t] * w[h],  w[h] = sum_g W_proj[g,h]

Sharding: 8 cores = 4 batches x 2 head-groups (8 heads each).

Per-core design (ScalarE-bound, everything else hidden under it):
  - Host pre-transposes g[b] and the head-group's W rows into e-major
    bf16 layouts (gT, wT) -> no on-device transposes at all.
  - Lift: q/k per head pair via PE matmuls (bf16 in, fp32 PSUM),
    DVE-copied to SBUF bf16; lift chunks are interleaved into the score
    stream only where PSUM ring banks are free.
  - Scores: per (head, q-tile of 128 rows) causal row block
    [0, 128*(qi+1)) matmul'd into a bank-aligned rotating PSUM ring;
    the two heads' matmuls are interleaved chunk-by-chunk so they run
    concurrently on disjoint PE row-groups (K=64 each). The causal mask
    of the diagonal block is ADDED by one extra matmul (lhsT=I,
    rhs=strictly-upper -30000).
  - Exp in place on PSUM (ScalarE); row-sums via accum_out for the big
    tiles (qi>=8) and via DVE reduce for the small ones (fewer
    ACTIVATION_READ_ACCUMULATOR instructions on the critical engine).
  - PE warm-up matmuls + dummy exp at t=0 keep HAM at 8/8 and preload
    the exp table during the input DMA.
  - Finale: lse = ln(sums) for all 128 (head, q-tile) columns at once
    via exponent/mantissa split; weighted head-sum via a replicated
    weight tile and 3 folding adds.
"""

import numpy as np
import ml_dtypes

B, T, E, H = 4, 2048, 1024, 16
D = 64
NCORES = 8
NE = 8         # 128-row chunks of e
PAIRS = 4      # head pairs per core
MASKV = -30000.0

_CACHE = {}


def _build():
    import concourse.bass as bass  # noqa: F401
    import concourse.tile as tile
    from concourse import bacc, mybir

    f32 = mybir.dt.float32
    bf16 = mybir.dt.bfloat16
    i32 = mybir.dt.int32
    EXP = mybir.ActivationFunctionType.Exp
    LN = mybir.ActivationFunctionType.Ln
    AX = mybir.AxisListType.X
    MUL = mybir.AluOpType.mult
    ADD = mybir.AluOpType.add
    SHR = mybir.AluOpType.logical_shift_right
    SUB = mybir.AluOpType.subtract
    AND = mybir.AluOpType.bitwise_and
    OR = mybir.AluOpType.bitwise_or
    LN2 = float(np.log(2.0))

    nc = bacc.Bacc("TRN2", target_bir_lowering=False, debug=False,
                   num_devices=NCORES)

    # gt rows: half*1024 + e*128 + p ; cols: t-within-half
    g_d = nc.dram_tensor("gt", [T, E], bf16, kind="ExternalInput").ap()
    # wt rows: pair*1024 + e*128 + p ; cols: f-within-pair (q0 q1 k0 k1)
    w_d = nc.dram_tensor("wt", [4 * E, 256], bf16, kind="ExternalInput").ap()
    tri_d = nc.dram_tensor("tri", [128, 128], bf16, kind="ExternalInput").ap()
    id_d = nc.dram_tensor("identb", [128, 128], bf16,
                          kind="ExternalInput").ap()
    wr_d = nc.dram_tensor("wrep", [128, 128], f32, kind="ExternalInput").ap()
    out_d = nc.dram_tensor("out_part", [128, 16], f32,
                           kind="ExternalOutput").ap()

    with tile.TileContext(nc) as tc:
        with (
            tc.tile_pool(name="consts", bufs=1) as consts,
            tc.tile_pool(name="big", bufs=1) as big,
            tc.tile_pool(name="qkp", bufs=2) as qkp,
            tc.tile_pool(name="sums", bufs=1) as sums,
            tc.tile_pool(name="fin", bufs=1) as fin,
            tc.tile_pool(name="ps", bufs=1, space="PSUM") as ps,
        ):
            # ---- constants (first: tiny, unblock warm-up) -----------------
            tri = consts.tile([128, 128], bf16, name="tri", tag="tri")
            nc.sync.dma_start(out=tri[:], in_=tri_d[:])
            identb = consts.tile([128, 128], bf16, name="identb", tag="id")
            nc.sync.dma_start(out=identb[:], in_=id_d[:])
            wrep = consts.tile([128, 128], f32, name="wrep", tag="wrep")
            nc.sync.dma_start(out=wrep[:], in_=wr_d[:])

            # ---- big SBUF layouts (host-pretransposed, bf16) --------------
            # gT col = e*2048 + t ; wT col = e*1024 + pair*256 + fw
            gT = big.tile([128, NE * T], bf16, name="gT", tag="gT")
            wT = big.tile([128, NE * E], bf16, name="wT", tag="wT")

            def dma_wt(pr):
                for e in range(NE):
                    nc.sync.dma_start(
                        out=wT[:, e * 1024 + pr * 256: e * 1024 + pr * 256 + 256],
                        in_=w_d[pr * 1024 + e * 128: pr * 1024 + e * 128 + 128, :])

            def dma_gt(tcn):
                # global t-chunk tcn in 0..3 -> (half, tc-within-half)
                half, tch = divmod(tcn, 2)
                for e in range(NE):
                    nc.sync.dma_start(
                        out=gT[:, e * 2048 + tcn * 512: e * 2048 + tcn * 512 + 512],
                        in_=g_d[half * 1024 + e * 128: half * 1024 + e * 128 + 128,
                                tch * 512: tch * 512 + 512])

            # critical-path DMAs first
            dma_wt(0)
            dma_gt(0)

            # ---- PSUM ring (8 banks of 512 fp32, bank-aligned regions) ----
            ring = ps.tile([128, 4096], f32, name="ring", tag="ring")
            ring_pos = [0]

            def ring_alloc(nbanks):
                if ring_pos[0] + nbanks > 8:
                    ring_pos[0] = 0
                off = ring_pos[0] * 512
                ring_pos[0] += nbanks
                return off

            # PE warm-up: ~3.5us of junk matmuls so HAM is at 8/8 when the
            # real lift starts; dummy exp preloads the ACT table set.
            joff = ring_alloc(1)
            for _ in range(28):
                nc.tensor.matmul(ring[:, joff:joff + 128], lhsT=identb[:],
                                 rhs=identb[:], start=True, stop=True)
            jx = fin.tile([128, 1], f32, name="jx", tag="jx")
            nc.scalar.activation(jx[:], wrep[:, 0:1], EXP)

            # rest of the input DMAs (overlap the first lift/score groups)
            dma_gt(1)
            dma_gt(2)
            dma_gt(3)
            for pr in range(1, PAIRS):
                dma_wt(pr)

            # sums[:, (2*pr+h)*16 + qi] = sum_j exp(scores) of that row tile
            sums1 = sums.tile([128, 128], f32, name="sums1", tag="sums")

            qkts = {}

            def lift_chunk(pr, ft, tcn):
                """qkt[pr] cols [ft*2048 + tcn*512, +512) from W f-tile."""
                off = ring_alloc(1)
                pt = ring[:, off:off + 512]
                w0 = pr * 256 + ft * 128
                for e in range(NE):
                    nc.tensor.matmul(
                        pt,
                        lhsT=wT[:, e * 1024 + w0: e * 1024 + w0 + 128],
                        rhs=gT[:, e * 2048 + tcn * 512:
                               e * 2048 + tcn * 512 + 512],
                        start=(e == 0), stop=(e == NE - 1))
                nc.vector.tensor_copy(
                    qkts[pr][:, ft * 2048 + tcn * 512:
                             ft * 2048 + tcn * 512 + 512], pt)

            def score_qtile(pr, qi):
                """Causal scores + exp/rowsum for both heads of pair pr,
                query rows [qi*128, +128). Head matmuls interleaved so they
                run concurrently on disjoint PE row-groups."""
                kneed = 128 * (qi + 1)
                nb = (kneed + 511) // 512
                qkt = qkts[pr]
                offs = [ring_alloc(nb), ring_alloc(nb)]
                lhsTs = [qkt[64 * h:64 * h + 64, qi * 128: qi * 128 + 128]
                         for h in range(2)]
                a = 0
                while a < kneed:
                    sz = min(512, kneed - a)
                    last = (a + sz == kneed)
                    for h in range(2):
                        nc.tensor.matmul(
                            ring[:, offs[h] + a: offs[h] + a + sz],
                            lhsT=lhsTs[h],
                            rhs=qkt[64 * h:64 * h + 64,
                                    2048 + a: 2048 + a + sz],
                            start=True, stop=not last)
                    a += sz
                for h in range(2):
                    # additive causal mask on the diagonal block
                    nc.tensor.matmul(
                        ring[:, offs[h] + kneed - 128: offs[h] + kneed],
                        lhsT=identb[:], rhs=tri[:], start=False, stop=True)
                for h in range(2):
                    col = (2 * pr + h) * 16 + qi
                    reg = ring[:, offs[h]: offs[h] + kneed]
                    if qi < 8:
                        nc.scalar.activation(reg, reg, EXP)
                        nc.vector.reduce_sum(out=sums1[:, col:col + 1],
                                             in_=reg, axis=AX)
                    else:
                        nc.scalar.activation(
                            reg, reg, EXP,
                            accum_out=sums1[:, col:col + 1])

            # ---- schedule -------------------------------------------------
            # lift chunks for pair p+1 go where ring banks are free:
            # after qi 4..7 (regions 1-2 banks) and qi 8..11 (3 banks).
            qkts[0] = qkp.tile([128, 2 * T], bf16, name="qkt0", tag="qkt")
            lift_chunk(0, 0, 0)
            lift_chunk(0, 1, 0)
            # pair 0 finishes its own lift early; pair 1's lift fills the
            # later slots (two per slot at qi 6/7).
            p0_self = {0: [(0, 0, 1)], 1: [(0, 1, 1)], 2: [(0, 0, 2)],
                       3: [(0, 1, 2)], 4: [(0, 0, 3)], 5: [(0, 1, 3)]}
            p0_next = {6: [(1, 0, 0), (1, 1, 0)], 7: [(1, 0, 1), (1, 1, 1)],
                       8: [(1, 0, 2)], 9: [(1, 1, 2)], 10: [(1, 0, 3)],
                       11: [(1, 1, 3)]}
            qkts[1] = qkp.tile([128, 2 * T], bf16, name="qkt1", tag="qkt")
            for qi in range(16):
                score_qtile(0, qi)
                for ch in p0_self.get(qi, []):
                    lift_chunk(*ch)
                for ch in p0_next.get(qi, []):
                    lift_chunk(*ch)

            order = [(0, 0), (1, 0), (0, 1), (1, 1),
                     (0, 2), (1, 2), (0, 3), (1, 3)]
            for pr in range(1, PAIRS):
                nxt = pr + 1
                if nxt < PAIRS:
                    qkts[nxt] = qkp.tile([128, 2 * T], bf16,
                                         name=f"qkt{nxt}", tag="qkt")
                for qi in range(16):
                    score_qtile(pr, qi)
                    if nxt < PAIRS and 4 <= qi < 12:
                        ft, tcn = order[qi - 4]
                        lift_chunk(nxt, ft, tcn)

            # ---- finale: lse = ln(sums); out = sum_h w[h]*lse_h -----------
            # s = m * 2^(e-127), m in [1,2):  lse = (e-127)*ln2 + Ln(m)
            u = sums1[:].bitcast(i32)
            ei = fin.tile([128, 128], i32, name="ei", tag="ei")
            nc.vector.tensor_scalar(out=ei[:], in0=u, scalar1=23,
                                    scalar2=None, op0=SHR)
            ef = fin.tile([128, 128], f32, name="ef", tag="ef")
            nc.vector.tensor_copy(ef[:], ei[:])
            nc.vector.tensor_scalar(out=ef[:], in0=ef[:], scalar1=127.0,
                                    scalar2=None, op0=SUB)
            mb = fin.tile([128, 128], i32, name="mb", tag="mb")
            nc.vector.tensor_scalar(out=mb[:], in0=u,
                                    scalar1=0x007FFFFF,
                                    scalar2=0x3F800000,
                                    op0=AND, op1=OR)
            lnm = fin.tile([128, 128], f32, name="lnm", tag="lnm")
            nc.scalar.activation(lnm[:], mb[:].bitcast(f32), LN)
            lse = fin.tile([128, 128], f32, name="lse", tag="lse")
            nc.vector.scalar_tensor_tensor(
                out=lse[:], in0=ef[:], scalar=LN2, in1=lnm[:],
                op0=MUL, op1=ADD)
            wl = fin.tile([128, 128], f32, name="wl", tag="wl")
            nc.vector.tensor_mul(wl[:], lse[:], wrep[:])
            h64 = fin.tile([128, 64], f32, name="h64", tag="h64")
            nc.vector.tensor_add(h64[:], wl[:, 0:64], wl[:, 64:128])
            h32 = fin.tile([128, 32], f32, name="h32", tag="h32")
            nc.vector.tensor_add(h32[:], h64[:, 0:32], h64[:, 32:64])
            facc = fin.tile([128, 16], f32, name="facc", tag="facc")
            nc.vector.tensor_add(facc[:], h32[:, 0:16], h32[:, 16:32])
            nc.sync.dma_start(out=out_d[:], in_=facc[:])

    nc.compile()
    return nc


def _get_nc():
    if "nc" not in _CACHE:
        _CACHE["nc"] = _build()
    return _CACHE["nc"]


def kernel(g, W_lift, W_proj):
    from concourse.bass_utils import run_bass_kernel_spmd

    bf16 = ml_dtypes.bfloat16
    g = np.asarray(g, dtype=np.float32)
    W_lift = np.asarray(W_lift, dtype=np.float32)
    W_proj = np.asarray(W_proj, dtype=np.float32)

    nc = _get_nc()
    w = W_proj.sum(axis=0).astype(np.float32)          # w[h] = sum_g W_proj[g,h]
    tri = np.triu(np.full((128, 128), MASKV, np.float32), k=1).astype(bf16)
    identb = np.eye(128, dtype=np.float32).astype(bf16)

    in_maps = []
    for core in range(NCORES):
        b, hg = core // 2, core % 2
        rows = []
        for p in range(PAIRS):
            h0 = hg * 8 + 2 * p
            h1 = h0 + 1
            rows += list(range(h0 * D, h0 * D + D))
            rows += list(range(h1 * D, h1 * D + D))
            rows += list(range(E + h0 * D, E + h0 * D + D))
            rows += list(range(E + h1 * D, E + h1 * D + D))
        W_slice = W_lift[rows, :]                      # [1024 f, 1024 e]
        wt_host = np.ascontiguousarray(
            W_slice.T.reshape(NE, 128, PAIRS, 256)
            .transpose(2, 0, 1, 3).reshape(4 * E, 256)).astype(bf16)
        gt_host = np.ascontiguousarray(
            g[b].reshape(2, 1024, NE, 128)
            .transpose(0, 2, 3, 1).reshape(T, E)).astype(bf16)
        wrep = np.ascontiguousarray(
            np.broadcast_to(np.repeat(w[hg * 8: hg * 8 + 8], 16)[None, :],
                            (128, 128))).astype(np.float32)
        in_maps.append({
            "gt": gt_host,
            "wt": wt_host,
            "tri": tri,
            "identb": identb,
            "wrep": wrep,
        })

    res = run_bass_kernel_spmd(nc, in_maps, core_ids=list(range(NCORES)))
    _CACHE["last_results"] = res
    _CACHE["last_in_maps"] = in_maps

    out = np.zeros((B, T), dtype=np.float32)
    for core in range(NCORES):
        b = core // 2
        part = res.results[core]["out_part"]           # (128, 16)
        out[b] += part.T.reshape(-1)
    return out
